# revision 3
# baseline (speedup 1.0000x reference)
"""LogicLayer Trainium2 kernel.

out[b, n] = sum_k softmax(w[n])_k * gate_k(a1, a2),  a1 = x[b, i1[n]], a2 = x[b, i2[n]]

All 16 differentiable gates are affine in {1, a1, a2, a1*a2}:
    out[b, n] = A0[n] + A1[n]*a1 + A2[n]*a2 + Ap[n]*a1*a2
with A* = softmax(w[n]) @ C for a constant [16, 4] table C.

Device plan (8 NeuronCores = 2 neuron-halves x 4 batch-quarters):
  - per core: neurons NN=4096 (half), batch NB=512 (quarter)
  - the core's x slice is shipped transposed (xt [8192, NB] f32) so that one
    neuron's input column is a contiguous 2KB row; dma_gather (SWDGE) pulls
    the two rows per neuron straight from HBM into SBUF, landing neuron-major
    tiles [128, slots, NB]
  - coefficients A0..Ap are computed on-device from w (ACT exp + DVE reduces)
  - inner loop is 2 fused DVE ops per slot:
        t   = (Ap*g2 + A1) * g1          (affine_mul_reduce)
        out = (A2*g2 + A0) + t           (affine_then_add)
  - output is written neuron-major [4096, NB]; host reassembles/transposes.
"""

import numpy as np

BATCH = 2048
NIN = 8192
NNEUR = 8192
NCORES = 8
N_NH = 2          # neuron halves
N_BQ = 4          # batch quarters
NN = NNEUR // N_NH    # neurons per core (4096)
NB = BATCH // N_BQ    # batch per core (512)
SLOTS = NN // 128     # 32
CHUNK_SLOTS = 8       # slots per gather call
NCHUNK = SLOTS // CHUNK_SLOTS  # 4
CHUNK_IDX = CHUNK_SLOTS * 128  # 1024 idxs per dma_gather

# gate -> (c0, c1, c2, cp) so gate_k(a1,a2) = c0 + c1*a1 + c2*a2 + cp*a1*a2
GATE_COEF = np.array(
    [
        [0, 0, 0, 0],    # FALSE
        [0, 0, 0, 1],    # AND
        [0, 1, 0, -1],   # a1 AND NOT a2
        [0, 1, 0, 0],    # a1
        [0, 0, 1, -1],   # NOT a1 AND a2
        [0, 0, 1, 0],    # a2
        [0, 1, 1, -2],   # XOR
        [0, 1, 1, -1],   # OR
        [1, -1, -1, 1],  # NOR
        [1, -1, -1, 2],  # XNOR
        [1, 0, -1, 0],   # NOT a2
        [1, 0, -1, 1],   # a1 OR NOT a2
        [1, -1, 0, 0],   # NOT a1
        [1, -1, 0, 1],   # NOT a1 OR a2
        [1, 0, 0, -1],   # NAND
        [1, 0, 0, 0],    # TRUE
    ],
    dtype=np.float32,
)  # [16, 4]

_CACHE = {}


def _build_nc():
    import concourse.bacc as bacc
    import concourse.mybir as mybir
    from concourse.tile import TileContext

    f32 = mybir.dt.float32
    i16 = mybir.dt.int16

    nc = bacc.Bacc("TRN2")
    xt = nc.dram_tensor("xt", [NIN, NB], f32, kind="ExternalInput")
    idx1 = nc.dram_tensor("idx1", [128, NN // 16], i16, kind="ExternalInput")
    idx2 = nc.dram_tensor("idx2", [128, NN // 16], i16, kind="ExternalInput")
    wr = nc.dram_tensor("wr", [128, SLOTS * 16], f32, kind="ExternalInput")
    ctab = nc.dram_tensor("ctab", [128, 4, SLOTS * 16], f32, kind="ExternalInput")
    yt = nc.dram_tensor("yt", [NN, NB], f32, kind="ExternalOutput")

    with TileContext(nc) as tc:
        with (
            tc.tile_pool(name="coef", bufs=1) as coef_pool,
            tc.tile_pool(name="work", bufs=3) as work_pool,
            tc.tile_pool(name="outp", bufs=3) as out_pool,
        ):
            # --- coefficients A0..Ap [128, SLOTS] from w ---
            wt = coef_pool.tile([128, SLOTS * 16], f32)
            nc.sync.dma_start(wt[:], wr[:])
            ct = coef_pool.tile([128, 4, SLOTS * 16], f32)
            nc.sync.dma_start(ct[:], ctab[:])
            it1 = coef_pool.tile([128, NN // 16], i16)
            nc.sync.dma_start(it1[:], idx1[:])
            it2 = coef_pool.tile([128, NN // 16], i16)
            nc.sync.dma_start(it2[:], idx2[:])

            ew = coef_pool.tile([128, SLOTS * 16], f32)
            nc.scalar.activation(ew[:], wt[:], mybir.ActivationFunctionType.Exp)
            ssum = coef_pool.tile([128, SLOTS], f32)
            nc.vector.tensor_reduce(
                ssum[:],
                ew[:].rearrange("p (s k) -> p s k", k=16),
                mybir.AxisListType.X,
                mybir.AluOpType.add,
            )
            rsum = coef_pool.tile([128, SLOTS], f32)
            nc.vector.reciprocal(rsum[:], ssum[:])

            acoef = coef_pool.tile([128, 4, SLOTS], f32)
            wtmp = coef_pool.tile([128, SLOTS * 16], f32)
            for c in range(4):
                nc.vector.tensor_mul(wtmp[:], ew[:], ct[:, c, :])
                nc.vector.tensor_reduce(
                    acoef[:, c, :],
                    wtmp[:].rearrange("p (s k) -> p s k", k=16),
                    mybir.AxisListType.X,
                    mybir.AluOpType.add,
                )
                nc.vector.tensor_mul(acoef[:, c, :], acoef[:, c, :], rsum[:])

            accum = coef_pool.tile([128, 1], f32)

            # --- main gather + combine loop ---
            for k in range(NCHUNK):
                g1 = work_pool.tile([128, CHUNK_SLOTS, NB], f32, tag="g1")
                nc.gpsimd.dma_gather(
                    g1[:], xt[:], it1[:, k * (CHUNK_IDX // 16):(k + 1) * (CHUNK_IDX // 16)],
                    CHUNK_IDX, CHUNK_IDX, NB,
                )
                g2 = work_pool.tile([128, CHUNK_SLOTS, NB], f32, tag="g2")
                nc.gpsimd.dma_gather(
                    g2[:], xt[:], it2[:, k * (CHUNK_IDX // 16):(k + 1) * (CHUNK_IDX // 16)],
                    CHUNK_IDX, CHUNK_IDX, NB,
                )
                ot = out_pool.tile([128, CHUNK_SLOTS, NB], f32, tag="ot")
                for s in range(CHUNK_SLOTS):
                    S = k * CHUNK_SLOTS + s
                    # t = (Ap*g2 + A1) * g1
                    nc.vector.affine_mul_reduce(
                        ot[:, s, :], accum[:],
                        g2[:, s, :], g1[:, s, :],
                        acoef[:, 3, S:S + 1], acoef[:, 1, S:S + 1],
                    )
                    # out = (A2*g2 + A0) + t
                    nc.vector.affine_then_add(
                        ot[:, s, :],
                        g2[:, s, :], ot[:, s, :],
                        acoef[:, 2, S:S + 1], acoef[:, 0, S:S + 1],
                    )
                dst = yt[k * CHUNK_IDX:(k + 1) * CHUNK_IDX, :].rearrange(
                    "(s p) b -> p s b", p=128
                )
                nc.sync.dma_start(dst, ot[:])

    nc.compile()
    return nc


def _prep_core_inputs(x, w, conn_indices):
    """Host-side shard/layout prep. Returns list of per-core input dicts."""
    maps = []
    for c in range(NCORES):
        nh, bq = divmod(c, N_BQ)
        n0 = nh * NN
        b0 = bq * NB
        xt = np.ascontiguousarray(x[b0:b0 + NB, :].T)  # [NIN, NB]

        def wrap(idx_list):
            # idx j lives at [j % 16, j // 16], replicated across 8 core groups
            wrapped = np.ascontiguousarray(idx_list.reshape(NN // 16, 16).T)
            return np.tile(wrapped, (8, 1)).astype(np.int16)

        i1 = conn_indices[n0:n0 + NN, 0].astype(np.int16)
        i2 = conn_indices[n0:n0 + NN, 1].astype(np.int16)
        # neuron n0 + s*128 + p  ->  partition p, slot s
        wslice = w[n0:n0 + NN, :].reshape(SLOTS, 128, 16).transpose(1, 0, 2)
        wr = np.ascontiguousarray(wslice.reshape(128, SLOTS * 16))
        ctab = np.ascontiguousarray(
            np.broadcast_to(
                GATE_COEF.T.reshape(1, 4, 1, 16), (128, 4, SLOTS, 16)
            ).reshape(128, 4, SLOTS * 16)
        )
        maps.append(
            {"xt": xt, "idx1": wrap(i1), "idx2": wrap(i2), "wr": wr, "ctab": ctab}
        )
    return maps


def run_cores(in_maps, trace=False):
    from concourse.bass_utils import run_bass_kernel_spmd

    if "nc" not in _CACHE:
        _CACHE["nc"] = _build_nc()
    return run_bass_kernel_spmd(
        _CACHE["nc"], in_maps, core_ids=list(range(NCORES)), trace=trace
    )


def _assemble(results):
    out = np.empty((BATCH, NNEUR), dtype=np.float32)
    for c in range(NCORES):
        nh, bq = divmod(c, N_BQ)
        n0 = nh * NN
        b0 = bq * NB
        out[b0:b0 + NB, n0:n0 + NN] = results[c]["yt"].T
    return out


def kernel(x, w, conn_indices):
    x = np.asarray(x, dtype=np.float32)
    w = np.asarray(w, dtype=np.float32)
    conn_indices = np.asarray(conn_indices)
    in_maps = _prep_core_inputs(x, w, conn_indices)
    res = run_cores(in_maps)
    return _assemble([r for r in res.results])


# revision 4
# speedup vs baseline: 1.3203x; 1.3203x over previous
"""LogicLayer Trainium2 kernel.

out[b, n] = sum_k softmax(w[n])_k * gate_k(a1, a2),  a1 = x[b, i1[n]], a2 = x[b, i2[n]]

All 16 differentiable gates are affine in {1, a1, a2, a1*a2}:
    out[b, n] = A0[n] + A1[n]*a1 + A2[n]*a2 + Ap[n]*a1*a2
with A* = softmax(w[n]) @ C for a constant [16, 4] table C.

Device plan (8 NeuronCores, neuron-sharded: 1024 neurons x full 2048 batch each):
  - x is shipped transposed (xt [8192, 2048] f32) so one neuron's input column
    is a contiguous 8KB row; dma_gather (SWDGE) pulls the two rows per neuron
    straight from HBM into SBUF, landing neuron-major tiles [128, slots, 2048].
    Fat 8KB descriptors keep the Q7 descriptor-generation cost (~8.5ns/desc)
    off the critical path (2048 descs/core).
  - coefficients A0..Ap are computed on-device from w (ACT exp + DVE reduces)
  - inner loop is 2 fused DVE ops per 128-neuron slot:
        t   = (Ap*g2 + A1) * g1          (affine_mul_reduce)
        out = (A2*g2 + A0) + t           (affine_then_add)
  - output is written neuron-major [1024, 2048]; host reassembles/transposes.
"""

import numpy as np

BATCH = 2048
NIN = 8192
NNEUR = 8192
NCORES = 8
NN = NNEUR // NCORES  # neurons per core (1024)
NB = BATCH            # full batch per core
SLOTS = NN // 128     # 8
CHUNK_SLOTS = 2       # slots per gather call
NCHUNK = SLOTS // CHUNK_SLOTS      # 4
CHUNK_IDX = CHUNK_SLOTS * 128      # 256 idxs per dma_gather

# gate -> (c0, c1, c2, cp) so gate_k(a1,a2) = c0 + c1*a1 + c2*a2 + cp*a1*a2
GATE_COEF = np.array(
    [
        [0, 0, 0, 0],    # FALSE
        [0, 0, 0, 1],    # AND
        [0, 1, 0, -1],   # a1 AND NOT a2
        [0, 1, 0, 0],    # a1
        [0, 0, 1, -1],   # NOT a1 AND a2
        [0, 0, 1, 0],    # a2
        [0, 1, 1, -2],   # XOR
        [0, 1, 1, -1],   # OR
        [1, -1, -1, 1],  # NOR
        [1, -1, -1, 2],  # XNOR
        [1, 0, -1, 0],   # NOT a2
        [1, 0, -1, 1],   # a1 OR NOT a2
        [1, -1, 0, 0],   # NOT a1
        [1, -1, 0, 1],   # NOT a1 OR a2
        [1, 0, 0, -1],   # NAND
        [1, 0, 0, 0],    # TRUE
    ],
    dtype=np.float32,
)  # [16, 4]

_CACHE = {}


def _build_nc():
    import concourse.bacc as bacc
    import concourse.mybir as mybir
    from concourse.tile import TileContext

    f32 = mybir.dt.float32
    i16 = mybir.dt.int16

    nc = bacc.Bacc("TRN2")
    xt = nc.dram_tensor("xt", [NIN, NB], f32, kind="ExternalInput")
    idx1 = nc.dram_tensor("idx1", [128, NN // 16], i16, kind="ExternalInput")
    idx2 = nc.dram_tensor("idx2", [128, NN // 16], i16, kind="ExternalInput")
    wr = nc.dram_tensor("wr", [128, SLOTS * 16], f32, kind="ExternalInput")
    ctab = nc.dram_tensor("ctab", [128, 4, SLOTS * 16], f32, kind="ExternalInput")
    yt = nc.dram_tensor("yt", [NN, NB], f32, kind="ExternalOutput")

    with TileContext(nc) as tc:
        with (
            tc.tile_pool(name="coef", bufs=1) as coef_pool,
            tc.tile_pool(name="work", bufs=3) as work_pool,
            tc.tile_pool(name="outp", bufs=2) as out_pool,
        ):
            # index tiles first so gathers can start immediately
            it1 = coef_pool.tile([128, NN // 16], i16)
            nc.sync.dma_start(it1[:], idx1[:])
            it2 = coef_pool.tile([128, NN // 16], i16)
            nc.sync.dma_start(it2[:], idx2[:])

            # --- coefficients A0..Ap [128, SLOTS] from w ---
            wt = coef_pool.tile([128, SLOTS * 16], f32)
            nc.sync.dma_start(wt[:], wr[:])
            ct = coef_pool.tile([128, 4, SLOTS * 16], f32)
            nc.sync.dma_start(ct[:], ctab[:])

            ew = coef_pool.tile([128, SLOTS * 16], f32)
            nc.scalar.activation(ew[:], wt[:], mybir.ActivationFunctionType.Exp)
            ssum = coef_pool.tile([128, SLOTS], f32)
            nc.vector.tensor_reduce(
                ssum[:],
                ew[:].rearrange("p (s k) -> p s k", k=16),
                mybir.AxisListType.X,
                mybir.AluOpType.add,
            )
            rsum = coef_pool.tile([128, SLOTS], f32)
            nc.vector.reciprocal(rsum[:], ssum[:])

            acoef = coef_pool.tile([128, 4, SLOTS], f32)
            wtmp = coef_pool.tile([128, SLOTS * 16], f32)
            for c in range(4):
                nc.vector.tensor_mul(wtmp[:], ew[:], ct[:, c, :])
                nc.vector.tensor_reduce(
                    acoef[:, c, :],
                    wtmp[:].rearrange("p (s k) -> p s k", k=16),
                    mybir.AxisListType.X,
                    mybir.AluOpType.add,
                )
                nc.vector.tensor_mul(acoef[:, c, :], acoef[:, c, :], rsum[:])

            accum = coef_pool.tile([128, 1], f32)

            # --- main gather + combine loop ---
            for k in range(NCHUNK):
                g1 = work_pool.tile([128, CHUNK_SLOTS, NB], f32, tag="g1")
                nc.gpsimd.dma_gather(
                    g1[:], xt[:],
                    it1[:, k * (CHUNK_IDX // 16):(k + 1) * (CHUNK_IDX // 16)],
                    CHUNK_IDX, CHUNK_IDX, NB,
                )
                g2 = work_pool.tile([128, CHUNK_SLOTS, NB], f32, tag="g2")
                nc.gpsimd.dma_gather(
                    g2[:], xt[:],
                    it2[:, k * (CHUNK_IDX // 16):(k + 1) * (CHUNK_IDX // 16)],
                    CHUNK_IDX, CHUNK_IDX, NB,
                )
                ot = out_pool.tile([128, CHUNK_SLOTS, NB], f32, tag="ot")
                for s in range(CHUNK_SLOTS):
                    S = k * CHUNK_SLOTS + s
                    # t = (Ap*g2 + A1) * g1
                    nc.vector.affine_mul_reduce(
                        ot[:, s, :], accum[:],
                        g2[:, s, :], g1[:, s, :],
                        acoef[:, 3, S:S + 1], acoef[:, 1, S:S + 1],
                    )
                    # out = (A2*g2 + A0) + t
                    nc.vector.affine_then_add(
                        ot[:, s, :],
                        g2[:, s, :], ot[:, s, :],
                        acoef[:, 2, S:S + 1], acoef[:, 0, S:S + 1],
                    )
                dst = yt[k * CHUNK_IDX:(k + 1) * CHUNK_IDX, :].rearrange(
                    "(s p) b -> p s b", p=128
                )
                nc.sync.dma_start(dst, ot[:])

    nc.compile()
    return nc


def _prep_core_inputs(x, w, conn_indices):
    """Host-side shard/layout prep. Returns list of per-core input dicts."""
    xt = np.ascontiguousarray(x.T)  # [NIN, BATCH], shared by all cores
    ctab = np.ascontiguousarray(
        np.broadcast_to(
            GATE_COEF.T.reshape(1, 4, 1, 16), (128, 4, SLOTS, 16)
        ).reshape(128, 4, SLOTS * 16)
    )
    maps = []
    for c in range(NCORES):
        n0 = c * NN

        def wrap(idx_list):
            # idx j lives at [j % 16, j // 16], replicated across 8 core groups
            wrapped = np.ascontiguousarray(idx_list.reshape(NN // 16, 16).T)
            return np.tile(wrapped, (8, 1)).astype(np.int16)

        i1 = conn_indices[n0:n0 + NN, 0].astype(np.int16)
        i2 = conn_indices[n0:n0 + NN, 1].astype(np.int16)
        # neuron n0 + s*128 + p  ->  partition p, slot s
        wslice = w[n0:n0 + NN, :].reshape(SLOTS, 128, 16).transpose(1, 0, 2)
        wr = np.ascontiguousarray(wslice.reshape(128, SLOTS * 16))
        maps.append(
            {"xt": xt, "idx1": wrap(i1), "idx2": wrap(i2), "wr": wr, "ctab": ctab}
        )
    return maps


def run_cores(in_maps, trace=False):
    from concourse.bass_utils import run_bass_kernel_spmd

    if "nc" not in _CACHE:
        _CACHE["nc"] = _build_nc()
    return run_bass_kernel_spmd(
        _CACHE["nc"], in_maps, core_ids=list(range(NCORES)), trace=trace
    )


def _assemble(results):
    out = np.empty((BATCH, NNEUR), dtype=np.float32)
    for c in range(NCORES):
        n0 = c * NN
        out[:, n0:n0 + NN] = results[c]["yt"].T
    return out


def kernel(x, w, conn_indices):
    x = np.asarray(x, dtype=np.float32)
    w = np.asarray(w, dtype=np.float32)
    conn_indices = np.asarray(conn_indices)
    in_maps = _prep_core_inputs(x, w, conn_indices)
    res = run_cores(in_maps)
    return _assemble([r for r in res.results])
